# revision 37
# baseline (speedup 1.0000x reference)
"""CRF mean-NLL kernel for Trainium2 (8 NeuronCores).

Problem: B=1024 sequences of length S=1024 with T=16 tags.
  nll = mean_b( logZ_b - gold_b )

Device strategy (SPMD, one uniform Bass/Tile program on 8 cores):
  - Sequence split 2-way: cores 0-3 run the FORWARD half (s in [0,512)),
    cores 4-7 run the BACKWARD half (s in [512,1024)); they meet at the
    midpoint and the (tiny) combine is a per-b dot product done on host.
  - Batch split 4-way: core c handles b-quarter q = c % 4 (256 rows).
  - Linear-domain recursion with the tag dimension on SBUF partitions,
    packed 8 groups x 16 tags = 128 partitions, 32 batch columns free:
        state <- (E8^T state) * u_t          (PE matmul + DVE multiply)
    where E8 = blockdiag(exp(transitions)) and u_t = exp(em_t - kappa).
    kappa = log(16) + 0.5 keeps magnitudes O(1) (deterministic log-shift,
    re-added on host), so no per-step renormalization is needed.
  - The chain is latency-bound (PE SBUF-access latency ~173 ns + DVE
    PSUM-access latency ~125 ns + semaphore props ~92 ns per step), so
    each core runs TWO independent 16-column chains whose
    (matmul -> multiply) rounds interleave on PE/DVE, hiding part of
    each other's fixed latency (~433 ns/step vs ~468 single-chain).
  - All gold-path terms (emission gather, transition pairs, start/end)
    depend only on tags + tiny tables and are summed on host.
"""

import os
import sys

import numpy as np

for _p in ("/opt/trn_rl_repo",):
    if os.path.isdir(_p) and _p not in sys.path:
        sys.path.insert(0, _p)

B, S, T = 1024, 1024, 16
NCORES = 8
G = 8                 # tag-groups packed on partitions
BB = 32               # batch columns per group (8*32 = 256 b per core)
BQ = G * BB           # 256 batch rows per core
SH = S // 2           # 512 steps per core
# u-chunks: small first chunk so the chain starts early
CHUNKS = [4, 60] + [64] * 7
assert sum(CHUNKS) == SH
KAPPA = float(np.log(16.0) + 0.5)
PRELOAD_W = os.environ.get("CRF_PRELOAD_W", "1") == "1"
NSPLIT = int(os.environ.get("CRF_NSPLIT", "2"))
assert BB % NSPLIT == 0

_PROGRAM = None
LAST_RESULTS = None   # BassKernelResults of the most recent run (for test.py)


def _build_program(trace_ready=False):
    """Build the uniform SPMD Bass program (compiled once, cached)."""
    global _PROGRAM
    if _PROGRAM is not None:
        return _PROGRAM

    import concourse.bacc as bacc
    import concourse.tile as tile
    from concourse import mybir

    f32 = mybir.dt.float32
    bf16 = mybir.dt.bfloat16
    Alu = mybir.AluOpType
    Act = mybir.ActivationFunctionType

    nc = bacc.Bacc(
        "TRN2",
        target_bir_lowering=False,
        debug=False,
        enable_asserts=False,
        num_devices=NCORES,
    )

    emlin = nc.dram_tensor("emlin", [128, SH * BB], bf16, kind="ExternalInput").ap()
    e8 = nc.dram_tensor("e8", [128, 128], bf16, kind="ExternalInput").ap()
    # consts[:, 0] = -kappa exp bias, consts[:, 1] = init state vector
    consts = nc.dram_tensor("consts", [128, 2], f32, kind="ExternalInput").ap()

    state_out = nc.dram_tensor("state", [128, BB], f32, kind="ExternalOutput").ap()

    with tile.TileContext(nc) as tc:
        with (
            tc.tile_pool(name="const", bufs=1) as constp,
            tc.tile_pool(name="emchunk", bufs=3) as emp,
            tc.tile_pool(name="u", bufs=len(CHUNKS)) as up,
            tc.tile_pool(name="state", bufs=3) as sp,
            tc.tile_pool(name="psum", bufs=4, space="PSUM") as pp,
        ):
            # dummy activation on a memset tile: pulls the EXP
            # ACT_TABLE_LOAD off the first real activation's critical path
            warm = constp.tile([128, 1], f32)
            warm2 = constp.tile([128, 1], f32)
            nc.gpsimd.memset(warm[:], 0.0)
            nc.scalar.activation(warm2[:], warm[:], Act.Exp)

            # DMA order: first emissions chunk, then tiny consts (exp bias +
            # init vector), then e8. DMA launch latency (~1 us) dominates all
            # three payloads, so the later-TRIGGERED transfer lands later —
            # trigger the big payload first and the 8-byte consts second.
            emc0 = emp.tile([128, CHUNKS[0] * BB], bf16, tag="emc")
            nc.sync.dma_start(emc0[:], emlin[:, 0:CHUNKS[0] * BB])
            cs_sb = constp.tile([128, 2], f32)
            nc.sync.dma_start(cs_sb[:], consts[:])
            kb_sb = cs_sb[:, 0:1]
            iv_sb = cs_sb[:, 1:2]
            e8_sb = constp.tile([128, 128], bf16)
            nc.sync.dma_start(e8_sb[:], e8[:])

            # bulk u = exp(em - kappa), chunked so the chain can start early
            u_tiles = []       # (tile, start_step, n_steps)
            s0 = 0
            for ci, csteps in enumerate(CHUNKS):
                cw = csteps * BB
                if ci == 0:
                    emc = emc0
                else:
                    emc = emp.tile([128, cw], bf16, tag="emc")
                    nc.sync.dma_start(emc[:], emlin[:, s0 * BB:(s0 + csteps) * BB])
                u_k = up.tile([128, cw], f32, tag="u")
                nc.scalar.activation(u_k[:], emc[:], Act.Exp, bias=kb_sb)
                u_tiles.append((u_k, s0, csteps))
                s0 += csteps

            def u_slice(t):
                for u_k, cs, cn in u_tiles:
                    if cs <= t < cs + cn:
                        off = (t - cs) * BB
                        return u_k[:, off:off + BB]
                raise AssertionError(t)

            # the stationary E8 never changes: load it once up front
            ldw = None
            if PRELOAD_W:
                ldw = nc.tensor.ldweights(e8_sb[:])

            # NSPLIT independent column-chains per core: chain h owns batch
            # columns [h*HB, (h+1)*HB). Their (matmul -> multiply) rounds
            # interleave on PE/DVE, hiding part of each other's fixed latency.
            HB = BB // NSPLIT
            states = []
            for h in range(NSPLIT):
                st = sp.tile([128, HB], bf16, tag=f"state{h}",
                             name=f"st0_{h}")[:]
                u0 = u_slice(0)
                nc.vector.tensor_scalar_mul(
                    st, u0[:, h * HB:(h + 1) * HB], iv_sb
                )
                states.append(st)

            # steps 1..511 of the recursion; the last step's multiplies land
            # in one f32 tile so a single DMA writes the final state out
            final = sp.tile([128, BB], f32, tag="final")
            first_mm = True
            for t in range(1, SH):
                last = t == SH - 1
                ut = u_slice(t)
                new_states = []
                for h in range(NSPLIT):
                    psb = pp.tile([128, HB], f32, tag=f"ps{h}",
                                  name=f"ps_{t}_{h}")
                    ps = psb[:]
                    mm = nc.tensor.matmul(
                        ps, e8_sb[:], states[h], start=True, stop=True,
                    )
                    if PRELOAD_W and first_mm:
                        mm.ins.add_dependency(
                            ldw.ins.name, mybir.DependencyInfo.SYNC_ONLY
                        )
                        first_mm = False
                    if last:
                        ns_ = final[:, h * HB:(h + 1) * HB]
                    else:
                        ns_ = sp.tile([128, HB], bf16, tag=f"state{h}",
                                      name=f"st_{t}_{h}")[:]
                    nc.vector.tensor_tensor(
                        ns_, ut[:, h * HB:(h + 1) * HB], ps, op=Alu.mult
                    )
                    new_states.append(ns_)
                states = new_states
            nc.sync.dma_start(state_out[:], final[:])

    nc.compile()
    _PROGRAM = nc
    return nc


def _host_prep(emissions, transitions, start_transitions, end_transitions):
    """Build the 8 per-core input dicts."""
    import ml_dtypes

    em = np.ascontiguousarray(emissions, dtype=np.float32)
    Tm = np.asarray(transitions, dtype=np.float64)
    E = np.exp(Tm)                       # E[i,j] = exp(trans[i,j])
    sv = np.exp(np.asarray(start_transitions, dtype=np.float64))
    ev = np.exp(np.asarray(end_transitions, dtype=np.float64))

    e8_f = np.zeros((128, 128), np.float32)
    e8_b = np.zeros((128, 128), np.float32)
    Ef32 = E.astype(np.float32)
    for g in range(G):
        e8_f[g * T:(g + 1) * T, g * T:(g + 1) * T] = Ef32
        e8_b[g * T:(g + 1) * T, g * T:(g + 1) * T] = Ef32.T
    e8_f = e8_f.astype(ml_dtypes.bfloat16)
    e8_b = e8_b.astype(ml_dtypes.bfloat16)

    in_maps = []
    for c in range(NCORES):
        fwd = c < 4
        q = c % 4
        emq = em[q * BQ:(q + 1) * BQ]                      # [256, 1024, 16]
        half = emq[:, :SH] if fwd else emq[:, SH:]         # [256, 512, 16]

        # chain layout [g, j, tau, bb]; bwd walks time reversed
        hh = half if fwd else half[:, ::-1]
        emlin = (
            hh.reshape(G, BB, SH, T)
            .transpose(0, 3, 2, 1)
            .reshape(128, SH * BB)
        )
        emlin = np.ascontiguousarray(emlin).astype(ml_dtypes.bfloat16)

        iv = sv if fwd else ev                             # [16]
        consts = np.empty((128, 2), np.float32)
        consts[:, 0] = -KAPPA
        consts[:, 1] = np.tile(iv, G)

        in_maps.append({
            "emlin": emlin,
            "e8": e8_f if fwd else e8_b,
            "consts": consts,
        })
    return in_maps, E


def _reference_numpy(emissions, tags, mask, transitions,
                     start_transitions, end_transitions):
    """Exact numpy replica of reference.py (fallback for unexpected inputs)."""
    em = np.asarray(emissions, dtype=np.float64)
    tg = np.asarray(tags).astype(np.int64)
    mk = np.asarray(mask).astype(bool)
    Tm = np.asarray(transitions, dtype=np.float64)
    sv = np.asarray(start_transitions, dtype=np.float64)
    ev = np.asarray(end_transitions, dtype=np.float64)
    Bn, Sn, Tn = em.shape

    bidx = np.arange(Bn)
    score = sv[tg[:, 0]] + em[bidx, 0, tg[:, 0]]
    emit = np.take_along_axis(em, tg[:, :, None], axis=2)[:, :, 0]
    trans = Tm[tg[:, 1:], tg[:, :-1]]
    m = mk[:, 1:].astype(np.float64)
    gold = score + np.sum((emit[:, 1:] + trans) * m, axis=1)
    last_idx = mk.astype(np.int64).sum(1) - 1
    last_tags = np.take_along_axis(tg, last_idx[:, None], axis=1)[:, 0]
    gold = gold + ev[last_tags]

    sc = sv[None, :] + em[:, 0]
    for t in range(1, Sn):
        nxt = sc[:, :, None] + Tm[None, :, :] + em[:, t][:, None, :]
        mx = nxt.max(axis=1)
        nxt = np.log(np.exp(nxt - mx[:, None, :]).sum(axis=1)) + mx
        sc = np.where(mk[:, t][:, None], nxt, sc)
    sc = sc + ev[None, :]
    mx = sc.max(axis=1)
    logZ = np.log(np.exp(sc - mx[:, None]).sum(axis=1)) + mx
    return np.float32(np.mean(logZ - gold))


def _ensure_ntff_hook():
    """Register the axon NTFF profile hook if the image lacks antenv.axon_hooks."""
    try:
        from antenv.axon_hooks import get_axon_ntff_profile_hook  # noqa: F401
        return
    except ImportError:
        pass
    import types
    try:
        import antenv
    except ImportError:
        antenv = types.ModuleType("antenv")
        sys.modules["antenv"] = antenv
    from trn_agent_boot.trn_boot import _ntff_profile_via_ctypes
    mod = types.ModuleType("antenv.axon_hooks")
    _state = {"h": None}
    mod.set_axon_ntff_profile_hook = lambda h: _state.__setitem__("h", h)
    mod.get_axon_ntff_profile_hook = lambda: _state["h"]
    sys.modules["antenv.axon_hooks"] = mod
    antenv.axon_hooks = mod
    h = _ntff_profile_via_ctypes("/opt/axon/libaxon_pjrt.so")
    if h is not None:
        mod.set_axon_ntff_profile_hook(h)


def kernel(emissions, tags, mask, transitions, start_transitions,
           end_transitions):
    global LAST_RESULTS
    emissions = np.asarray(emissions)
    tags = np.asarray(tags)
    mask = np.asarray(mask)
    transitions = np.asarray(transitions)
    start_transitions = np.asarray(start_transitions)
    end_transitions = np.asarray(end_transitions)

    if (emissions.shape != (B, S, T)) or not bool(np.all(mask)):
        return _reference_numpy(emissions, tags, mask, transitions,
                                start_transitions, end_transitions)

    import concourse.bass_utils as bass_utils
    from concourse.bass_utils import run_bass_kernel_spmd

    nc = _build_program()
    in_maps, E = _host_prep(emissions, transitions,
                            start_transitions, end_transitions)

    trace = os.environ.get("CRF_TRACE", "0") == "1"
    kw = {}
    if trace:
        _ensure_ntff_hook()
        bass_utils.upload_artifacts = lambda d: f"local:{d}"
        kw["tmpdir"] = os.environ.get("CRF_TRACE_DIR") or None
    res = run_bass_kernel_spmd(nc, in_maps, list(range(NCORES)), trace=trace, **kw)
    LAST_RESULTS = res

    # ---- host combine (tiny) ----
    em32 = np.asarray(emissions, dtype=np.float32)
    tg = tags.astype(np.int64)
    Tm = np.asarray(transitions, dtype=np.float64)
    sv = np.asarray(start_transitions, dtype=np.float64)
    ev = np.asarray(end_transitions, dtype=np.float64)

    logZ = np.empty(B, np.float64)
    for q in range(4):
        a = res.results[q]["state"].astype(np.float64).reshape(G, T, BB)
        sbk = res.results[q + 4]["state"].astype(np.float64).reshape(G, T, BB)
        bvec = np.einsum("ij,gjb->gib", E, sbk)        # E @ s = beta_511
        z = np.einsum("gib,gib->gb", a, bvec)          # [G, BB]
        logZ[q * BQ:(q + 1) * BQ] = (
            np.log(z) + (2 * SH) * KAPPA
        ).reshape(BQ)                                  # b = g*32+bb order

    emit = np.take_along_axis(em32, tg[:, :, None], axis=2)[:, :, 0]  # [B,S]
    gold = (
        emit.sum(axis=1, dtype=np.float64)
        + sv[tg[:, 0]]
        + ev[tg[:, -1]]
        + Tm[tg[:, 1:], tg[:, :-1]].sum(axis=1)
    )
    return np.float32(np.mean(logZ - gold))


# revision 39
# speedup vs baseline: 1.0087x; 1.0087x over previous
"""CRF mean-NLL kernel for Trainium2 (8 NeuronCores).

Problem: B=1024 sequences of length S=1024 with T=16 tags.
  nll = mean_b( logZ_b - gold_b )

Device strategy (SPMD, one uniform Bass/Tile program on 8 cores):
  - Sequence split 2-way: cores 0-3 run the FORWARD half (s in [0,512)),
    cores 4-7 run the BACKWARD half (s in [512,1024)); they meet at the
    midpoint and the (tiny) combine is a per-b dot product done on host.
  - Batch split 4-way: core c handles b-quarter q = c % 4 (256 rows).
  - Linear-domain recursion with the tag dimension on SBUF partitions,
    packed 8 groups x 16 tags = 128 partitions, 32 batch columns free:
        state <- (E8^T state) * u_t          (PE matmul + DVE multiply)
    where E8 = blockdiag(exp(transitions)) and u_t = exp(em_t - kappa).
    kappa = log(16) + 0.5 keeps magnitudes O(1) (deterministic log-shift,
    re-added on host), so no per-step renormalization is needed.
  - The chain is latency-bound (PE SBUF-access latency ~173 ns + DVE
    PSUM-access latency ~125 ns + semaphore props ~92 ns per step), so
    each core runs TWO independent 16-column chains whose
    (matmul -> multiply) rounds interleave on PE/DVE, hiding part of
    each other's fixed latency (~433 ns/step vs ~468 single-chain).
  - All gold-path terms (emission gather, transition pairs, start/end)
    depend only on tags + tiny tables and are summed on host.
"""

import os
import sys

import numpy as np

for _p in ("/opt/trn_rl_repo",):
    if os.path.isdir(_p) and _p not in sys.path:
        sys.path.insert(0, _p)

B, S, T = 1024, 1024, 16
NCORES = 8
G = 8                 # tag-groups packed on partitions
BB = 32               # batch columns per group (8*32 = 256 b per core)
BQ = G * BB           # 256 batch rows per core
SH = S // 2           # 512 steps per core
# u-chunks: small first chunk so the chain starts early
CHUNKS = [8, 56] + [64] * 7
assert sum(CHUNKS) == SH
KAPPA = float(np.log(16.0) + 0.5)
PRELOAD_W = os.environ.get("CRF_PRELOAD_W", "1") == "1"
NSPLIT = int(os.environ.get("CRF_NSPLIT", "2"))
assert BB % NSPLIT == 0

_PROGRAM = None
LAST_RESULTS = None   # BassKernelResults of the most recent run (for test.py)


def _build_program(trace_ready=False):
    """Build the uniform SPMD Bass program (compiled once, cached)."""
    global _PROGRAM
    if _PROGRAM is not None:
        return _PROGRAM

    import concourse.bacc as bacc
    import concourse.tile as tile
    from concourse import mybir

    f32 = mybir.dt.float32
    bf16 = mybir.dt.bfloat16
    Alu = mybir.AluOpType
    Act = mybir.ActivationFunctionType

    nc = bacc.Bacc(
        "TRN2",
        target_bir_lowering=False,
        debug=False,
        enable_asserts=False,
        num_devices=NCORES,
    )

    emlin = nc.dram_tensor("emlin", [128, SH * BB], bf16, kind="ExternalInput").ap()
    e8 = nc.dram_tensor("e8", [128, 128], bf16, kind="ExternalInput").ap()
    # consts[:, 0] = -kappa exp bias, consts[:, 1] = init state vector
    consts = nc.dram_tensor("consts", [128, 2], f32, kind="ExternalInput").ap()

    state_out = nc.dram_tensor("state", [128, BB], f32, kind="ExternalOutput").ap()

    with tile.TileContext(nc) as tc:
        with (
            tc.tile_pool(name="const", bufs=1) as constp,
            tc.tile_pool(name="emchunk", bufs=3) as emp,
            tc.tile_pool(name="u", bufs=len(CHUNKS)) as up,
            tc.tile_pool(name="state", bufs=3) as sp,
            tc.tile_pool(name="psum", bufs=4, space="PSUM") as pp,
        ):
            # dummy activation on a memset tile: pulls the EXP
            # ACT_TABLE_LOAD off the first real activation's critical path
            warm = constp.tile([128, 1], f32)
            warm2 = constp.tile([128, 1], f32)
            nc.gpsimd.memset(warm[:], 0.0)
            nc.scalar.activation(warm2[:], warm[:], Act.Exp)

            # DMA order: first emissions chunk, then tiny consts, then e8.
            # Launch latency (~1 us) dominates all payloads, so the
            # later-TRIGGERED transfer lands later — trigger the big payload
            # first and the 8-byte consts second.
            emc0 = emp.tile([128, CHUNKS[0] * BB], bf16, tag="emc")
            nc.sync.dma_start(emc0[:], emlin[:, 0:CHUNKS[0] * BB])
            cs_sb = constp.tile([128, 2], f32)
            nc.sync.dma_start(cs_sb[:], consts[:])
            kb_sb = cs_sb[:, 0:1]
            iv_sb = cs_sb[:, 1:2]
            e8_sb = constp.tile([128, 128], bf16)
            nc.sync.dma_start(e8_sb[:], e8[:])

            # bulk u = exp(em - kappa), chunked so the chain can start early
            u_tiles = []       # (tile, start_step, n_steps)
            s0 = 0
            for ci, csteps in enumerate(CHUNKS):
                cw = csteps * BB
                if ci == 0:
                    emc = emc0
                else:
                    emc = emp.tile([128, cw], bf16, tag="emc")
                    nc.sync.dma_start(emc[:], emlin[:, s0 * BB:(s0 + csteps) * BB])
                u_k = up.tile([128, cw], f32, tag="u")
                nc.scalar.activation(u_k[:], emc[:], Act.Exp, bias=kb_sb)
                u_tiles.append((u_k, s0, csteps))
                s0 += csteps

            def u_slice(t):
                for u_k, cs, cn in u_tiles:
                    if cs <= t < cs + cn:
                        off = (t - cs) * BB
                        return u_k[:, off:off + BB]
                raise AssertionError(t)

            # the stationary E8 never changes: load it once up front
            ldw = None
            if PRELOAD_W:
                ldw = nc.tensor.ldweights(e8_sb[:])

            # NSPLIT independent column-chains per core: chain h owns batch
            # columns [h*HB, (h+1)*HB). Their (matmul -> multiply) rounds
            # interleave on PE/DVE, hiding part of each other's fixed latency.
            HB = BB // NSPLIT
            states = []
            for h in range(NSPLIT):
                st = sp.tile([128, HB], bf16, tag=f"state{h}",
                             name=f"st0_{h}")[:]
                u0 = u_slice(0)
                nc.vector.tensor_scalar_mul(
                    st, u0[:, h * HB:(h + 1) * HB], iv_sb
                )
                states.append(st)

            # steps 1..511 of the recursion; the last step's multiplies land
            # in one f32 tile so a single DMA writes the final state out
            final = sp.tile([128, BB], f32, tag="final")
            first_mm = True
            for t in range(1, SH):
                last = t == SH - 1
                ut = u_slice(t)
                new_states = []
                for h in range(NSPLIT):
                    psb = pp.tile([128, HB], f32, tag=f"ps{h}",
                                  name=f"ps_{t}_{h}")
                    ps = psb[:]
                    mm = nc.tensor.matmul(
                        ps, e8_sb[:], states[h], start=True, stop=True,
                    )
                    if PRELOAD_W and first_mm:
                        mm.ins.add_dependency(
                            ldw.ins.name, mybir.DependencyInfo.SYNC_ONLY
                        )
                        first_mm = False
                    if last:
                        ns_ = final[:, h * HB:(h + 1) * HB]
                    else:
                        ns_ = sp.tile([128, HB], bf16, tag=f"state{h}",
                                      name=f"st_{t}_{h}")[:]
                    nc.vector.tensor_tensor(
                        ns_, ut[:, h * HB:(h + 1) * HB], ps, op=Alu.mult
                    )
                    new_states.append(ns_)
                states = new_states
            nc.sync.dma_start(state_out[:], final[:])

    nc.compile()
    _PROGRAM = nc
    return nc


def _host_prep(emissions, transitions, start_transitions, end_transitions):
    """Build the 8 per-core input dicts."""
    import ml_dtypes

    em = np.ascontiguousarray(emissions, dtype=np.float32)
    Tm = np.asarray(transitions, dtype=np.float64)
    E = np.exp(Tm)                       # E[i,j] = exp(trans[i,j])
    sv = np.exp(np.asarray(start_transitions, dtype=np.float64))
    ev = np.exp(np.asarray(end_transitions, dtype=np.float64))

    e8_f = np.zeros((128, 128), np.float32)
    e8_b = np.zeros((128, 128), np.float32)
    Ef32 = E.astype(np.float32)
    for g in range(G):
        e8_f[g * T:(g + 1) * T, g * T:(g + 1) * T] = Ef32
        e8_b[g * T:(g + 1) * T, g * T:(g + 1) * T] = Ef32.T
    e8_f = e8_f.astype(ml_dtypes.bfloat16)
    e8_b = e8_b.astype(ml_dtypes.bfloat16)

    in_maps = []
    for c in range(NCORES):
        fwd = c < 4
        q = c % 4
        emq = em[q * BQ:(q + 1) * BQ]                      # [256, 1024, 16]
        half = emq[:, :SH] if fwd else emq[:, SH:]         # [256, 512, 16]

        # chain layout [g, j, tau, bb]; bwd walks time reversed
        hh = half if fwd else half[:, ::-1]
        emlin = (
            hh.reshape(G, BB, SH, T)
            .transpose(0, 3, 2, 1)
            .reshape(128, SH * BB)
        )
        emlin = np.ascontiguousarray(emlin).astype(ml_dtypes.bfloat16)

        iv = sv if fwd else ev                             # [16]
        consts = np.empty((128, 2), np.float32)
        consts[:, 0] = -KAPPA
        consts[:, 1] = np.tile(iv, G)

        in_maps.append({
            "emlin": emlin,
            "e8": e8_f if fwd else e8_b,
            "consts": consts,
        })
    return in_maps, E


def _reference_numpy(emissions, tags, mask, transitions,
                     start_transitions, end_transitions):
    """Exact numpy replica of reference.py (fallback for unexpected inputs)."""
    em = np.asarray(emissions, dtype=np.float64)
    tg = np.asarray(tags).astype(np.int64)
    mk = np.asarray(mask).astype(bool)
    Tm = np.asarray(transitions, dtype=np.float64)
    sv = np.asarray(start_transitions, dtype=np.float64)
    ev = np.asarray(end_transitions, dtype=np.float64)
    Bn, Sn, Tn = em.shape

    bidx = np.arange(Bn)
    score = sv[tg[:, 0]] + em[bidx, 0, tg[:, 0]]
    emit = np.take_along_axis(em, tg[:, :, None], axis=2)[:, :, 0]
    trans = Tm[tg[:, 1:], tg[:, :-1]]
    m = mk[:, 1:].astype(np.float64)
    gold = score + np.sum((emit[:, 1:] + trans) * m, axis=1)
    last_idx = mk.astype(np.int64).sum(1) - 1
    last_tags = np.take_along_axis(tg, last_idx[:, None], axis=1)[:, 0]
    gold = gold + ev[last_tags]

    sc = sv[None, :] + em[:, 0]
    for t in range(1, Sn):
        nxt = sc[:, :, None] + Tm[None, :, :] + em[:, t][:, None, :]
        mx = nxt.max(axis=1)
        nxt = np.log(np.exp(nxt - mx[:, None, :]).sum(axis=1)) + mx
        sc = np.where(mk[:, t][:, None], nxt, sc)
    sc = sc + ev[None, :]
    mx = sc.max(axis=1)
    logZ = np.log(np.exp(sc - mx[:, None]).sum(axis=1)) + mx
    return np.float32(np.mean(logZ - gold))


def _ensure_ntff_hook():
    """Register the axon NTFF profile hook if the image lacks antenv.axon_hooks."""
    try:
        from antenv.axon_hooks import get_axon_ntff_profile_hook  # noqa: F401
        return
    except ImportError:
        pass
    import types
    try:
        import antenv
    except ImportError:
        antenv = types.ModuleType("antenv")
        sys.modules["antenv"] = antenv
    from trn_agent_boot.trn_boot import _ntff_profile_via_ctypes
    mod = types.ModuleType("antenv.axon_hooks")
    _state = {"h": None}
    mod.set_axon_ntff_profile_hook = lambda h: _state.__setitem__("h", h)
    mod.get_axon_ntff_profile_hook = lambda: _state["h"]
    sys.modules["antenv.axon_hooks"] = mod
    antenv.axon_hooks = mod
    h = _ntff_profile_via_ctypes("/opt/axon/libaxon_pjrt.so")
    if h is not None:
        mod.set_axon_ntff_profile_hook(h)


def kernel(emissions, tags, mask, transitions, start_transitions,
           end_transitions):
    global LAST_RESULTS
    emissions = np.asarray(emissions)
    tags = np.asarray(tags)
    mask = np.asarray(mask)
    transitions = np.asarray(transitions)
    start_transitions = np.asarray(start_transitions)
    end_transitions = np.asarray(end_transitions)

    if (emissions.shape != (B, S, T)) or not bool(np.all(mask)):
        return _reference_numpy(emissions, tags, mask, transitions,
                                start_transitions, end_transitions)

    import concourse.bass_utils as bass_utils
    from concourse.bass_utils import run_bass_kernel_spmd

    nc = _build_program()
    in_maps, E = _host_prep(emissions, transitions,
                            start_transitions, end_transitions)

    trace = os.environ.get("CRF_TRACE", "0") == "1"
    kw = {}
    if trace:
        _ensure_ntff_hook()
        bass_utils.upload_artifacts = lambda d: f"local:{d}"
        kw["tmpdir"] = os.environ.get("CRF_TRACE_DIR") or None
    res = run_bass_kernel_spmd(nc, in_maps, list(range(NCORES)), trace=trace, **kw)
    LAST_RESULTS = res

    # ---- host combine (tiny) ----
    em32 = np.asarray(emissions, dtype=np.float32)
    tg = tags.astype(np.int64)
    Tm = np.asarray(transitions, dtype=np.float64)
    sv = np.asarray(start_transitions, dtype=np.float64)
    ev = np.asarray(end_transitions, dtype=np.float64)

    logZ = np.empty(B, np.float64)
    for q in range(4):
        a = res.results[q]["state"].astype(np.float64).reshape(G, T, BB)
        sbk = res.results[q + 4]["state"].astype(np.float64).reshape(G, T, BB)
        bvec = np.einsum("ij,gjb->gib", E, sbk)        # E @ s = beta_511
        z = np.einsum("gib,gib->gb", a, bvec)          # [G, BB]
        logZ[q * BQ:(q + 1) * BQ] = (
            np.log(z) + (2 * SH) * KAPPA
        ).reshape(BQ)                                  # b = g*32+bb order

    emit = np.take_along_axis(em32, tg[:, :, None], axis=2)[:, :, 0]  # [B,S]
    gold = (
        emit.sum(axis=1, dtype=np.float64)
        + sv[tg[:, 0]]
        + ev[tg[:, -1]]
        + Tm[tg[:, 1:], tg[:, :-1]].sum(axis=1)
    )
    return np.float32(np.mean(logZ - gold))
